# revision 9
# baseline (speedup 1.0000x reference)
"""MHSA kernel for 8 Trainium2 NeuronCores (Bass/Tile).

Distribution (per sharding hint): data-parallel over batch (4) x
tensor-parallel over heads (2 groups of 8 heads) = 8 shards, one per core.

Per core (batch b, head-group t), on device:
  - transpose x[b] via PE -> xT (bf16)
  - qT = wqT.T @ xT, kT = wkT.T @ xT  (head-transposed layouts)
  - v  = x @ wv.T in natural [n, dv] layout, augmented with a ones column
  - per (head, 512-wide n-chunk): scoresT tiles [m=128, n=512] = kT.T @ qT,
    exp on ScalarE (no max subtraction: scores ~ N(0,1)), PV matmul with
    stationary [v_h | ones] accumulating outT[c(64)+denom(1), n] in PSUM,
    normalize by the broadcast reciprocal denominator -> vhT (bf16)
  - out_part = vhT.T @ woT  [2048, 1024] bf16
Host sums the two TP partials per batch (fp32) and adds the bias.

Executed via the bass2jax/PJRT path (axon) on cores 0-7; the compiled
sharded callable is cached module-level so repeated calls are fast.
"""
import numpy as np
import ml_dtypes

import concourse.bacc as bacc
import concourse.mybir as mybir
import concourse.tile as tile
from concourse.masks import make_identity

B, N, C, H = 4, 2048, 1024, 16
HD = C // H          # 64
NCORES = 8
TP = 2               # head groups
HPG = H // TP        # 8 heads per group
DPG = HPG * HD       # 512 dims per group
SCALE = HD ** -0.5

F32 = mybir.dt.float32
BF16 = mybir.dt.bfloat16
BF = ml_dtypes.bfloat16

_cache = {}


def _build_nc():
    nc = bacc.Bacc("TRN2", num_devices=NCORES)
    # sharded inputs: each core ships 1/2 of x[b] and 1/4 of its weight set;
    # on-device AllGathers reassemble them (saves host->device wire bytes).
    xh_d = nc.declare_dram_parameter("xh", [N // 2, C], BF16, isOutput=False)
    wq4_d = nc.declare_dram_parameter("wq4", [C // 4, 3 * DPG], BF16, isOutput=False)
    wo4_d = nc.declare_dram_parameter("wo4", [DPG // 4, C], BF16, isOutput=False)
    # output: this core's half of the TP-summed batch output (pair ReduceScatter)
    out_d = nc.declare_dram_parameter("out", [N // 2, C], BF16, isOutput=True)

    xh_i = nc.dram_tensor("xh_i", [N // 2, C], BF16)
    x_d = nc.dram_tensor("x_g", [N, C], BF16)
    wq4_i = nc.dram_tensor("wq4_i", [C // 4, 3 * DPG], BF16)
    wqkv_d = nc.dram_tensor("wqkv_g", [C, 3 * DPG], BF16)
    wo4_i = nc.dram_tensor("wo4_i", [DPG // 4, C], BF16)
    wo_d = nc.dram_tensor("wo_g", [DPG, C], BF16)
    outp_i = nc.dram_tensor("outp_i", [N, C], BF16)
    out_rs = nc.dram_tensor("out_rs", [N // 2, C], BF16)

    PAIRS = [[2 * b, 2 * b + 1] for b in range(B)]
    TGROUPS = [[0, 2, 4, 6], [1, 3, 5, 7]]

    CS = C // 128     # 8 c-slices
    NB = N // 128     # 16 n-blocks
    NCH = N // 512    # 4 n-chunks
    DB = DPG // 128   # 4 d-blocks for qT/kT and dv-slices for vhT

    with tile.TileContext(nc) as tc:
        with (
            tc.tile_pool(name="big", bufs=1) as big,
            tc.tile_pool(name="weights", bufs=1) as wpool,
        ):
            # persistent SBUF tensors
            xT = big.tile([128, CS, N], BF16)          # x.T  (c, n)
            qT = big.tile([128, DB, N], BF16)          # q.T  (d, n), scale folded
            kT = big.tile([128, DB, N], BF16)
            v = big.tile([128, NB, HPG, 72], BF16)     # v natural + ones col @64
            vhT = big.tile([128, DB, N], BF16)         # normalized attn output .T
            wqkv = wpool.tile([128, CS, 3 * DPG], BF16)
            wo = wpool.tile([128, DB, C], BF16)
            ident = wpool.tile([128, 128], BF16)
            ones65 = wpool.tile([65, 64], F32)

            make_identity(nc, ident[:, :])
            nc.vector.memset(ones65[64:65, :], 1.0)
            nc.vector.memset(v[:, :, :, 64:65], 1.0)

            # reassemble sharded inputs on device
            nc.gpsimd.dma_start(out=xh_i[:], in_=xh_d[:])
            nc.gpsimd.collective_compute(
                "AllGather", mybir.AluOpType.bypass, replica_groups=PAIRS,
                ins=[xh_i[:]], outs=[x_d[:]])
            nc.gpsimd.dma_start(out=wq4_i[:], in_=wq4_d[:])
            nc.gpsimd.collective_compute(
                "AllGather", mybir.AluOpType.bypass, replica_groups=TGROUPS,
                ins=[wq4_i[:]], outs=[wqkv_d[:]])
            nc.gpsimd.dma_start(out=wo4_i[:], in_=wo4_d[:])
            nc.gpsimd.collective_compute(
                "AllGather", mybir.AluOpType.bypass, replica_groups=TGROUPS,
                ins=[wo4_i[:]], outs=[wo_d[:]])

            for cs in range(CS):
                nc.sync.dma_start(out=wqkv[:, cs, :],
                                  in_=wqkv_d[cs * 128:(cs + 1) * 128, :])
            for ds in range(DB):
                nc.sync.dma_start(out=wo[:, ds, :], in_=wo_d[ds * 128:(ds + 1) * 128, :])

            # ---- phase A: load x, transpose on PE, project q/k/v ----
            with (
                tc.tile_pool(name="xin", bufs=3) as xin,
                tc.tile_pool(name="tp_ps", bufs=4, space="PSUM") as tp_ps,
                tc.tile_pool(name="pj_ps", bufs=2, space="PSUM") as pj_ps,
            ):
                for nb in range(NB):
                    xn = xin.tile([128, C], BF16)
                    nc.sync.dma_start(out=xn[:, :], in_=x_d[nb * 128:(nb + 1) * 128, :])
                    for cs in range(CS):
                        tp = tp_ps.tile([128, 128], BF16)
                        nc.tensor.transpose(tp[:, :], xn[:, cs * 128:(cs + 1) * 128],
                                            ident[:, :])
                        nc.vector.tensor_copy(xT[:, cs, nb * 128:(nb + 1) * 128], tp[:, :])

                # qT/kT: [d-block 128, n-chunk 512] accumulating over c-slices
                for qk in range(2):
                    dst = (qT, kT)[qk]
                    for db in range(DB):
                        for ch in range(NCH):
                            ps = pj_ps.tile([128, 512], F32)
                            for cs in range(CS):
                                nc.tensor.matmul(
                                    ps[:, :],
                                    wqkv[:, cs, qk * DPG + db * 128:
                                         qk * DPG + (db + 1) * 128],
                                    xT[:, cs, ch * 512:(ch + 1) * 512],
                                    start=(cs == 0), stop=(cs == CS - 1))
                            nc.scalar.copy(dst[:, db, ch * 512:(ch + 1) * 512], ps[:, :])

                # v natural: [n-block 128, dv 512] accumulating over c-slices
                for nb in range(NB):
                    ps = pj_ps.tile([128, HPG, 64], F32)
                    for cs in range(CS):
                        nc.tensor.matmul(
                            ps[:, :, :],
                            xT[:, cs, nb * 128:(nb + 1) * 128],
                            wqkv[:, cs, 2 * DPG:3 * DPG],
                            start=(cs == 0), stop=(cs == CS - 1))
                    nc.scalar.copy(v[:, nb, :, 0:64], ps[:, :, :])

            # ---- phase B: attention per (head, n-chunk) ----
            with (
                tc.tile_pool(name="st_ps", bufs=3, space="PSUM") as st_ps,
                tc.tile_pool(name="pv_ps", bufs=2, space="PSUM") as pv_ps,
                tc.tile_pool(name="bc_ps", bufs=2, space="PSUM") as bc_ps,
                tc.tile_pool(name="est", bufs=3) as est_pool,
                tc.tile_pool(name="small", bufs=4) as small,
            ):
                for h in range(HPG):
                    po = (h % 2) * 64          # partition offset of this head
                    db = h // 2                # d-block holding this head
                    for ch in range(NCH):
                        qh = qT[po:po + 64, db, ch * 512:(ch + 1) * 512]
                        pv = pv_ps.tile([65, 512], F32)
                        for mb in range(NB):
                            st = st_ps.tile([128, 512], F32)
                            nc.tensor.matmul(
                                st[:, :],
                                kT[po:po + 64, db, mb * 128:(mb + 1) * 128],
                                qh, start=True, stop=True)
                            est = est_pool.tile([128, 512], BF16)
                            nc.scalar.activation(est[:, :], st[:, :],
                                                 mybir.ActivationFunctionType.Exp)
                            nc.tensor.matmul(
                                pv[:, :], v[:, mb, h, 0:65], est[:, :],
                                start=(mb == 0), stop=(mb == NB - 1))
                        # denominator -> SBUF, broadcast over 64 partitions on PE
                        den = small.tile([65, 512], F32)
                        nc.vector.tensor_copy(den[64:65, :], pv[64:65, :])
                        bc = bc_ps.tile([64, 512], F32)
                        nc.tensor.matmul(bc[:, :], ones65[64:65, :], den[64:65, :],
                                         start=True, stop=True)
                        rec = small.tile([64, 512], F32)
                        nc.vector.reciprocal_approx_fast(rec[:, :], bc[:, :])
                        vh = small.tile([64, 512], BF16)
                        nc.vector.tensor_mul(vh[:, :], pv[0:64, :], rec[:, :])
                        nc.sync.dma_start(
                            out=vhT[po:po + 64, db, ch * 512:(ch + 1) * 512],
                            in_=vh[:, :])

            # ---- phase C: output projection ----
            with (
                tc.tile_pool(name="op_ps", bufs=4, space="PSUM") as op_ps,
                tc.tile_pool(name="osb", bufs=3) as osb,
            ):
                for nb in range(NB):
                    ot = osb.tile([128, C], BF16)
                    for dch in range(2):
                        ps = op_ps.tile([128, 512], F32)
                        for ds in range(DB):
                            nc.tensor.matmul(
                                ps[:, :],
                                vhT[:, ds, nb * 128:(nb + 1) * 128],
                                wo[:, ds, dch * 512:(dch + 1) * 512],
                                start=(ds == 0), stop=(ds == DB - 1))
                        nc.scalar.copy(ot[:, dch * 512:(dch + 1) * 512], ps[:, :])
                    nc.sync.dma_start(out=outp_i[nb * 128:(nb + 1) * 128, :],
                                      in_=ot[:, :])

            # TP partial sum on device: pair ReduceScatter, each core keeps
            # its half of the summed batch output
            nc.gpsimd.collective_compute(
                "ReduceScatter", mybir.AluOpType.add, replica_groups=PAIRS,
                ins=[outp_i[:]], outs=[out_rs[:]])
            nc.gpsimd.dma_start(out=out_d[:], in_=out_rs[:])

    nc.compile()
    return nc


def _get_nc():
    if "nc" not in _cache:
        _cache["nc"] = _build_nc()
    return _cache["nc"]


def _prep(x, w_qkv, w_out):
    """Build per-core input maps. Core c -> (batch c//2, head-group c%2).

    Each core ships: half of x[b] (its pair-AllGather contribution), and a
    quarter of its head-group's weights (4-core-group AllGather).
    """
    x = np.asarray(x, dtype=np.float32)
    w_qkv = np.asarray(w_qkv, dtype=np.float32)
    w_out = np.asarray(w_out, dtype=np.float32)
    w_q, w_k, w_v = w_qkv[0:C], w_qkv[C:2 * C], w_qkv[2 * C:3 * C]
    xb = [x[b].astype(BF) for b in range(B)]
    per_t = []
    for t in range(TP):
        sl = slice(t * DPG, (t + 1) * DPG)
        wqkvT = np.concatenate(
            [(w_q[sl] * SCALE).T, w_k[sl].T, w_v[sl].T], axis=1)
        per_t.append((
            np.ascontiguousarray(wqkvT).astype(BF),        # [C, 3*DPG]
            np.ascontiguousarray(w_out[:, sl].T).astype(BF),  # [DPG, C]
        ))
    in_maps = []
    for c in range(NCORES):
        b, t = c // TP, c % TP
        g = c // 2          # index of this core within its weight group
        wqkvT, woT = per_t[t]
        in_maps.append({
            "xh": xb[b][t * (N // 2):(t + 1) * (N // 2)],
            "wq4": wqkvT[g * (C // 4):(g + 1) * (C // 4)],
            "wo4": woT[g * (DPG // 4):(g + 1) * (DPG // 4)],
        })
    return in_maps


def _get_runner():
    """Persistent jitted SPMD callable over the 8 cores (built once).

    Mirrors concourse.bass2jax.run_bass_via_pjrt, but caches the jitted
    function, and does NOT pass zero buffers for the outputs: this kernel
    writes every element of its output, so the outputs can be plain
    custom-call results (saves shipping zeros per call).
    """
    if "runner" in _cache:
        return _cache["runner"]
    import jax
    import numpy as _np
    from jax.sharding import Mesh, PartitionSpec
    from jax.experimental.shard_map import shard_map
    from concourse import bass2jax, mybir as _mybir

    nc = _get_nc()
    bass2jax.install_neuronx_cc_hook()

    partition_name = nc.partition_id_tensor.name if nc.partition_id_tensor else None
    in_names, out_names, out_avals = [], [], []
    for alloc in nc.m.functions[0].allocations:
        if not isinstance(alloc, _mybir.MemoryLocationSet):
            continue
        name = alloc.memorylocations[0].name
        if alloc.kind == "ExternalInput":
            if name != partition_name:
                in_names.append(name)
        elif alloc.kind == "ExternalOutput":
            shape = tuple(alloc.tensor_shape)
            dtype = _mybir.dt.np(alloc.dtype)
            out_names.append(name)
            out_avals.append(jax.core.ShapedArray(shape, dtype))
    n_params = len(in_names)
    all_names = list(in_names)
    if partition_name is not None:
        all_names.append(partition_name)

    def _body(*args):
        operands = list(args)
        if partition_name is not None:
            operands.append(bass2jax.partition_id_tensor())
        outs = bass2jax._bass_exec_p.bind(
            *operands,
            out_avals=tuple(out_avals),
            in_names=tuple(all_names),
            out_names=tuple(out_names),
            lowering_input_output_aliases=(),
            sim_require_finite=True,
            sim_require_nnan=True,
            nc=nc,
        )
        return tuple(outs)

    devices = jax.devices()[:NCORES]
    mesh = Mesh(_np.asarray(devices), ("core",))
    sharded = jax.jit(
        shard_map(
            _body, mesh=mesh,
            in_specs=(PartitionSpec("core"),) * n_params,
            out_specs=(PartitionSpec("core"),) * len(out_names),
            check_rep=False,
        ),
        keep_unused=True,
    )

    def prepare(in_maps):
        return [
            _np.concatenate([_np.asarray(m[name]) for m in in_maps], axis=0)
            for name in in_names
        ]

    def gather(out_arrs):
        return [
            {
                name: _np.asarray(out_arrs[i]).reshape(
                    NCORES, *out_avals[i].shape)[c]
                for i, name in enumerate(out_names)
            }
            for c in range(NCORES)
        ]

    _cache["runner"] = (sharded, prepare, gather)
    return _cache["runner"]


def kernel(x, w_qkv, w_out, b_out):
    b_out = np.asarray(b_out, dtype=np.float32)
    sharded, prepare, gather = _get_runner()
    import jax
    ops = prepare(_prep(x, w_qkv, w_out))
    res = gather(jax.block_until_ready(sharded(*ops)))
    out = np.empty((B, N, C), dtype=np.float32)
    for b in range(B):
        out[b, 0:N // 2] = res[2 * b]["out"].astype(np.float32) + b_out[None, :]
        out[b, N // 2:N] = res[2 * b + 1]["out"].astype(np.float32) + b_out[None, :]
    return out


if __name__ == "__main__":
    rng = np.random.default_rng(0)
    x = rng.standard_normal((B, N, C)).astype(np.float32)
    w_qkv = (rng.standard_normal((3 * C, C)) * C ** -0.5).astype(np.float32)
    w_out = (rng.standard_normal((C, C)) * C ** -0.5).astype(np.float32)
    b_out = (rng.standard_normal(C) * 0.01).astype(np.float32)
    got = kernel(x=x, w_qkv=w_qkv, w_out=w_out, b_out=b_out)

    # numpy reference
    q = x @ (w_qkv[0:C]).T * SCALE
    k = x @ (w_qkv[C:2 * C]).T
    v = x @ (w_qkv[2 * C:]).T
    def heads(t):
        return t.reshape(B, N, H, HD).transpose(0, 2, 1, 3)
    qh, kh, vh = heads(q), heads(k), heads(v)
    s = np.einsum("bhnc,bhmc->bhnm", qh, kh)
    s = np.exp(s - s.max(-1, keepdims=True))
    a = s / s.sum(-1, keepdims=True)
    o = np.einsum("bhnm,bhmc->bhnc", a, vh).transpose(0, 2, 1, 3).reshape(B, N, C)
    exp = o @ w_out.T + b_out
    err = np.abs(got - exp).max() / np.abs(exp).max()
    print("rel err:", err)


# revision 21
# speedup vs baseline: 2.9148x; 2.9148x over previous
"""MHSA kernel for 8 Trainium2 NeuronCores (Bass/Tile).

Distribution (per sharding hint): data-parallel over batch (4) x
tensor-parallel over heads (2 groups of 8 heads) = 8 shards, one per core.

Per core (batch b, head-group t), on device:
  - transpose x[b] via PE -> xT (bf16)
  - qT = wqT.T @ xT, kT = wkT.T @ xT  (head-transposed layouts)
  - v  = x @ wv.T in natural [n, dv] layout, augmented with a ones column
  - per (head, 512-wide n-chunk): scoresT tiles [m=128, n=512] = kT.T @ qT,
    exp on ScalarE (no max subtraction: scores ~ N(0,1)), PV matmul with
    stationary [v_h | ones] accumulating outT[c(64)+denom(1), n] in PSUM,
    normalize by the broadcast reciprocal denominator -> vhT (bf16)
  - out_part = vhT.T @ woT  [2048, 1024] bf16
Host sums the two TP partials per batch (fp32) and adds the bias.

Executed via the bass2jax/PJRT path (axon) on cores 0-7; the compiled
sharded callable is cached module-level so repeated calls are fast.
"""
import numpy as np
import ml_dtypes

import concourse.bacc as bacc
import concourse.mybir as mybir
import concourse.tile as tile
from concourse.masks import make_identity

B, N, C, H = 4, 2048, 1024, 16
HD = C // H          # 64
NCORES = 8
TP = 2               # head groups
HPG = H // TP        # 8 heads per group
DPG = HPG * HD       # 512 dims per group
SCALE = HD ** -0.5

F32 = mybir.dt.float32
BF16 = mybir.dt.bfloat16
BF = ml_dtypes.bfloat16

_cache = {}


USE_CC = True   # on-device collectives for input reassembly / output TP-sum


def _build_nc(use_cc=None):
    if use_cc is None:
        use_cc = USE_CC
    nc = bacc.Bacc("TRN2", num_devices=NCORES)

    CS = C // 128     # 8 c-slices
    NB = N // 128     # 16 n-blocks
    NCH = N // 512    # 4 n-chunks
    DB = DPG // 128   # 4 d-blocks for qT/kT and dv-slices for vhT
    NQ = 4            # row quarters for the x-AllGather / out-ReduceScatter

    PAIRS = [[2 * b, 2 * b + 1] for b in range(B)]
    TGROUPS = [[0, 2, 4, 6], [1, 3, 5, 7]]

    if use_cc:
        # sharded inputs: each core ships 1/2 of x[b] and 1/4 of its weight
        # set; on-device AllGathers reassemble them (saves host wire bytes).
        # x is gathered in row quarters so compute overlaps the collective.
        # single packed input operand (per-operand dispatch cost is high):
        # rows [0:1024] xh, [1024:1280] wqk quarter, [1280:1408] wv quarter
        # (as [128, 1024]), [1408:1536] wo quarter
        inp_d = nc.declare_dram_parameter("inp", [3 * N // 4, C], BF16,
                                          isOutput=False)
        # output: this core's share of the TP-summed batch output, as 4
        # quarter ReduceScatters (rows q*256:(q+1)*256 of out_d = global
        # rows q*512+t*256 : q*512+(t+1)*256 of the batch output)
        out_d = nc.declare_dram_parameter("out", [N // 2, C], BF16, isOutput=True)
        xh_i = nc.dram_tensor("xh_i", [N // 2, C], BF16)
        x_d = nc.dram_tensor("x_g", [N, C], BF16)
        wqk4_i = nc.dram_tensor("wqk4_i", [C // 4, 2 * DPG], BF16)
        # wv quarter is [256, 512] semantically; declared [128, 1024] to
        # match the packed-input row shape (same bytes, row-major)
        wqk_g = nc.dram_tensor("wqk_g", [C, 2 * DPG], BF16)
        wv4_i = nc.dram_tensor("wv4_i", [C // 8, C], BF16)
        wv_g = nc.dram_tensor("wv_g", [C, DPG], BF16)
        wo4_i = nc.dram_tensor("wo4_i", [DPG // 4, C], BF16)
        wo_d = nc.dram_tensor("wo_g", [DPG, C], BF16)
        outp_q = [nc.dram_tensor(f"outp_q{q}", [N // 4, C], BF16)
                  for q in range(NQ)]
        out_rsq = [nc.dram_tensor(f"out_rs{q}", [N // 8, C], BF16)
                   for q in range(NQ)]
    else:
        x_d = nc.declare_dram_parameter("x", [N, C], BF16, isOutput=False)
        wqkv_d = nc.declare_dram_parameter("wqkvT", [C, 3 * DPG], BF16,
                                           isOutput=False)
        wo_d = nc.declare_dram_parameter("woT", [DPG, C], BF16, isOutput=False)
        out_full = nc.declare_dram_parameter("out", [N, C], BF16, isOutput=True)

    with tile.TileContext(nc) as tc:
        with (
            tc.tile_pool(name="big", bufs=1) as big,
            tc.tile_pool(name="weights", bufs=1) as wpool,
        ):
            # persistent SBUF tensors (separate tiles per slice so the Tile
            # scheduler tracks dependencies at fine grain and can overlap
            # later projections with early attention)
            xT = [big.tile([128, N], BF16, name=f"xT{cs}", tag=f"xT{cs}") for cs in range(CS)]
            qTl = [big.tile([128, N], BF16, name=f"qT{db}", tag=f"qT{db}") for db in range(DB)]
            kTl = [big.tile([128, N], BF16, name=f"kT{db}", tag=f"kT{db}") for db in range(DB)]
            v = big.tile([128, NB, HPG, 72], BF16)     # v natural + ones col @64
            vhTc = [[big.tile([128, 512], BF16, name=f"vhT{ds}_{ch}",
                              tag=f"vhT{ds}_{ch}") for ch in range(NCH)]
                    for ds in range(DB)]
            wqk_sb = [wpool.tile([128, 2 * DPG], BF16, name=f"wqk{cs}",
                                 tag=f"wqk{cs}") for cs in range(CS)]
            wv_sb = [wpool.tile([128, DPG], BF16, name=f"wv{cs}",
                                tag=f"wv{cs}") for cs in range(CS)]
            wo = wpool.tile([128, DB, C], BF16)
            ident = wpool.tile([128, 128], BF16)
            ones65 = wpool.tile([65, 64], F32)

            make_identity(nc, ident[:, :])
            nc.vector.memset(ones65[64:65, :], 1.0)
            nc.vector.memset(v[:, :, :, 64:65], 1.0)

            if use_cc:
                # reassemble sharded inputs on device; order matters: x first
                # (transposes), then wv (v-projection), then wqk, then wo
                nc.gpsimd.dma_start(out=xh_i[:], in_=inp_d[0:N // 2, :])
                nc.gpsimd.collective_compute(
                    "AllGather", mybir.AluOpType.bypass, replica_groups=PAIRS,
                    ins=[xh_i[:]], outs=[x_d[:]])
                nc.gpsimd.dma_start(out=wv4_i[:], in_=inp_d[1280:1408, :])
                nc.gpsimd.collective_compute(
                    "AllGather", mybir.AluOpType.bypass, replica_groups=TGROUPS,
                    ins=[wv4_i[:]], outs=[wv_g[:]])
                nc.gpsimd.dma_start(out=wqk4_i[:], in_=inp_d[1024:1280, :])
                nc.gpsimd.collective_compute(
                    "AllGather", mybir.AluOpType.bypass, replica_groups=TGROUPS,
                    ins=[wqk4_i[:]], outs=[wqk_g[:]])
                nc.gpsimd.dma_start(out=wo4_i[:], in_=inp_d[1408:1536, :])
                nc.gpsimd.collective_compute(
                    "AllGather", mybir.AluOpType.bypass, replica_groups=TGROUPS,
                    ins=[wo4_i[:]], outs=[wo_d[:]])
                for cs in range(CS):
                    nc.sync.dma_start(out=wv_sb[cs][:, :],
                                      in_=wv_g[cs * 128:(cs + 1) * 128, :])
                    nc.sync.dma_start(out=wqk_sb[cs][:, :],
                                      in_=wqk_g[cs * 128:(cs + 1) * 128, :])
            else:
                for cs in range(CS):
                    nc.sync.dma_start(
                        out=wv_sb[cs][:, :],
                        in_=wqkv_d[cs * 128:(cs + 1) * 128, 2 * DPG:3 * DPG])
                    nc.sync.dma_start(
                        out=wqk_sb[cs][:, :],
                        in_=wqkv_d[cs * 128:(cs + 1) * 128, 0:2 * DPG])
            for ds in range(DB):
                nc.sync.dma_start(out=wo[:, ds, :],
                                  in_=wo_d[ds * 128:(ds + 1) * 128, :])

            xsrc = [(nb, x_d, nb * 128) for nb in range(NB)]

            # ---- phase A: load x, transpose on PE, project v then q/k ----
            with (
                tc.tile_pool(name="xin", bufs=3) as xin,
                tc.tile_pool(name="tp_ps", bufs=4, space="PSUM") as tp_ps,
                tc.tile_pool(name="pj_ps", bufs=2, space="PSUM") as pj_ps,
            ):
                for nb, src, row in xsrc:
                    xn = xin.tile([128, C], BF16)
                    nc.sync.dma_start(out=xn[:, :], in_=src[row:row + 128, :])
                    for cs in range(CS):
                        tp = tp_ps.tile([128, 128], BF16)
                        nc.tensor.transpose(tp[:, :], xn[:, cs * 128:(cs + 1) * 128],
                                            ident[:, :])
                        nc.vector.tensor_copy(xT[cs][:, nb * 128:(nb + 1) * 128],
                                              tp[:, :])

                # v natural first (attention consumes all of v): [n-block 128,
                # dv 512] accumulating over c-slices
                for nb in range(NB):
                    ps = pj_ps.tile([128, HPG, 64], F32)
                    for cs in range(CS):
                        nc.tensor.matmul(
                            ps[:, :, :],
                            xT[cs][:, nb * 128:(nb + 1) * 128],
                            wv_sb[cs][:, :],
                            start=(cs == 0), stop=(cs == CS - 1))
                    nc.vector.tensor_copy(v[:, nb, :, 0:64], ps[:, :, :])

                # qT/kT per d-block (attention head h needs d-block h//2)
                for db in range(DB):
                    for qk in range(2):
                        dst = (qTl, kTl)[qk][db]
                        for ch in range(NCH):
                            ps = pj_ps.tile([128, 512], F32, tag="psqk")
                            for cs in range(CS):
                                nc.tensor.matmul(
                                    ps[:, :],
                                    wqk_sb[cs][:, qk * DPG + db * 128:
                                               qk * DPG + (db + 1) * 128],
                                    xT[cs][:, ch * 512:(ch + 1) * 512],
                                    start=(cs == 0), stop=(cs == CS - 1))
                            nc.vector.tensor_copy(
                                dst[:, ch * 512:(ch + 1) * 512], ps[:, :])

            # ---- phase B+C: attention (ch outer) with the output
            # projection and quarter ReduceScatter interleaved per n-chunk ----
            with (
                tc.tile_pool(name="st_ps", bufs=2, space="PSUM") as st_ps,
                tc.tile_pool(name="pv_ps", bufs=2, space="PSUM") as pv_ps,
                tc.tile_pool(name="mix_ps", bufs=2, space="PSUM") as mix_ps,
                tc.tile_pool(name="est", bufs=3) as est_pool,
                tc.tile_pool(name="small", bufs=4) as small,
                tc.tile_pool(name="osb", bufs=3) as osb,
            ):
                for ch in range(NCH):
                    for h in range(HPG):
                        po = (h % 2) * 64      # partition offset of this head
                        db = h // 2            # d-block holding this head
                        qh = qTl[db][po:po + 64, ch * 512:(ch + 1) * 512]
                        pv = pv_ps.tile([65, 512], F32)
                        for mb in range(0, NB, 2):
                            # two m-blocks per PSUM tile -> one 1024-wide exp
                            st = st_ps.tile([128, 2, 512], F32)
                            est = est_pool.tile([128, 2, 512], BF16)
                            for j in range(2):
                                nc.tensor.matmul(
                                    st[:, j, :],
                                    kTl[db][po:po + 64,
                                            (mb + j) * 128:(mb + j + 1) * 128],
                                    qh, start=True, stop=True)
                            nc.scalar.activation(est[:, :, :], st[:, :, :],
                                                 mybir.ActivationFunctionType.Exp)
                            for j in range(2):
                                nc.tensor.matmul(
                                    pv[:, :], v[:, mb + j, h, 0:65], est[:, j, :],
                                    start=(mb + j == 0), stop=(mb + j == NB - 1))
                        # denominator -> SBUF, broadcast over 64 partitions (PE)
                        den = small.tile([65, 512], F32)
                        nc.vector.tensor_copy(den[64:65, :], pv[64:65, :])
                        bc = mix_ps.tile([64, 512], F32, tag="mix")
                        nc.tensor.matmul(bc[:, :], ones65[64:65, :], den[64:65, :],
                                         start=True, stop=True)
                        rec = small.tile([64, 512], F32)
                        nc.vector.reciprocal_approx_fast(rec[:, :], bc[:, :])
                        vh = small.tile([64, 512], BF16)
                        nc.vector.tensor_mul(vh[:, :], pv[0:64, :], rec[:, :])
                        nc.sync.dma_start(
                            out=vhTc[db][ch][po:po + 64, :], in_=vh[:, :])

                    # output projection for this n-chunk (4 n-blocks)
                    for nb in range(4 * ch, 4 * ch + 4):
                        ot = osb.tile([128, C], BF16)
                        for dch in range(2):
                            ps = mix_ps.tile([128, 512], F32, tag="mix")
                            for ds in range(DB):
                                nc.tensor.matmul(
                                    ps[:, :],
                                    vhTc[ds][ch][:, (nb - 4 * ch) * 128:
                                                 (nb - 4 * ch + 1) * 128],
                                    wo[:, ds, dch * 512:(dch + 1) * 512],
                                    start=(ds == 0), stop=(ds == DB - 1))
                            nc.vector.tensor_copy(
                                ot[:, dch * 512:(dch + 1) * 512], ps[:, :])
                        if use_cc:
                            nc.sync.dma_start(
                                out=outp_q[ch][(nb % 4) * 128:(nb % 4 + 1) * 128, :],
                                in_=ot[:, :])
                        else:
                            nc.sync.dma_start(
                                out=out_full[nb * 128:(nb + 1) * 128, :],
                                in_=ot[:, :])
                    if use_cc:
                        # this quarter is complete: TP partial sum on device,
                        # each core keeps its half of the quarter
                        nc.gpsimd.collective_compute(
                            "ReduceScatter", mybir.AluOpType.add,
                            replica_groups=PAIRS,
                            ins=[outp_q[ch][:]], outs=[out_rsq[ch][:]])
                        nc.gpsimd.dma_start(
                            out=out_d[ch * (N // 8):(ch + 1) * (N // 8), :],
                            in_=out_rsq[ch][:])

    nc.compile()
    return nc


def _get_nc():
    if "nc" not in _cache:
        _cache["nc"] = _build_nc()
    return _cache["nc"]


def _prep(x, w_qkv, w_out):
    """Build per-core input maps. Core c -> (batch c//2, head-group c%2).

    Each core ships: half of x[b] (its pair-AllGather contribution), and a
    quarter of its head-group's weights (4-core-group AllGather).
    """
    x = np.asarray(x, dtype=np.float32)
    w_qkv = np.asarray(w_qkv, dtype=np.float32)
    w_out = np.asarray(w_out, dtype=np.float32)
    w_q, w_k, w_v = w_qkv[0:C], w_qkv[C:2 * C], w_qkv[2 * C:3 * C]
    xb = [x[b].astype(BF) for b in range(B)]
    per_t = []
    for t in range(TP):
        sl = slice(t * DPG, (t + 1) * DPG)
        wqkvT = np.concatenate(
            [(w_q[sl] * SCALE).T, w_k[sl].T, w_v[sl].T], axis=1)
        per_t.append((
            np.ascontiguousarray(wqkvT).astype(BF),        # [C, 3*DPG]
            np.ascontiguousarray(w_out[:, sl].T).astype(BF),  # [DPG, C]
        ))
    in_maps = []
    for c in range(NCORES):
        b, t = c // TP, c % TP
        g = c // 2          # index of this core within its weight group
        wqkvT, woT = per_t[t]
        if USE_CC:
            inp = np.empty((3 * N // 4, C), dtype=BF)
            inp[0:N // 2] = xb[b][t * (N // 2):(t + 1) * (N // 2)]
            inp[1024:1280] = wqkvT[g * 256:(g + 1) * 256, 0:1024]
            inp[1280:1408] = wqkvT[g * 256:(g + 1) * 256,
                                   1024:1536].reshape(128, 1024)
            inp[1408:1536] = woT[g * 128:(g + 1) * 128]
            in_maps.append({"inp": inp})
        else:
            in_maps.append({"x": xb[b], "wqkvT": wqkvT, "woT": woT})
    return in_maps


def _get_runner():
    """Persistent jitted SPMD callable over the 8 cores (built once).

    Mirrors concourse.bass2jax.run_bass_via_pjrt, but caches the jitted
    function, and does NOT pass zero buffers for the outputs: this kernel
    writes every element of its output, so the outputs can be plain
    custom-call results (saves shipping zeros per call).
    """
    if "runner" in _cache:
        return _cache["runner"]
    import jax
    import numpy as _np
    from jax.sharding import Mesh, PartitionSpec
    from jax.experimental.shard_map import shard_map
    from concourse import bass2jax, mybir as _mybir

    nc = _get_nc()
    bass2jax.install_neuronx_cc_hook()

    partition_name = nc.partition_id_tensor.name if nc.partition_id_tensor else None
    in_names, out_names, out_avals = [], [], []
    for alloc in nc.m.functions[0].allocations:
        if not isinstance(alloc, _mybir.MemoryLocationSet):
            continue
        name = alloc.memorylocations[0].name
        if alloc.kind == "ExternalInput":
            if name != partition_name:
                in_names.append(name)
        elif alloc.kind == "ExternalOutput":
            shape = tuple(alloc.tensor_shape)
            dtype = _mybir.dt.np(alloc.dtype)
            out_names.append(name)
            out_avals.append(jax.core.ShapedArray(shape, dtype))
    n_params = len(in_names)
    all_names = list(in_names)
    if partition_name is not None:
        all_names.append(partition_name)

    def _body(*args):
        operands = list(args)
        if partition_name is not None:
            operands.append(bass2jax.partition_id_tensor())
        outs = bass2jax._bass_exec_p.bind(
            *operands,
            out_avals=tuple(out_avals),
            in_names=tuple(all_names),
            out_names=tuple(out_names),
            lowering_input_output_aliases=(),
            sim_require_finite=True,
            sim_require_nnan=True,
            nc=nc,
        )
        return tuple(outs)

    devices = jax.devices()[:NCORES]
    mesh = Mesh(_np.asarray(devices), ("core",))
    sharded = jax.jit(
        shard_map(
            _body, mesh=mesh,
            in_specs=(PartitionSpec("core"),) * n_params,
            out_specs=(PartitionSpec("core"),) * len(out_names),
            check_rep=False,
        ),
        keep_unused=True,
    )

    def prepare(in_maps):
        return [
            _np.concatenate([_np.asarray(m[name]) for m in in_maps], axis=0)
            for name in in_names
        ]

    def gather(out_arrs):
        return [
            {
                name: _np.asarray(out_arrs[i]).reshape(
                    NCORES, *out_avals[i].shape)[c]
                for i, name in enumerate(out_names)
            }
            for c in range(NCORES)
        ]

    _cache["runner"] = (sharded, prepare, gather)
    return _cache["runner"]


def kernel(x, w_qkv, w_out, b_out):
    b_out = np.asarray(b_out, dtype=np.float32)
    sharded, prepare, gather = _get_runner()
    import jax
    ops = prepare(_prep(x, w_qkv, w_out))
    res = gather(jax.block_until_ready(sharded(*ops)))
    out = np.empty((B, N, C), dtype=np.float32)
    for b in range(B):
        if USE_CC:
            # quarter-RS layout: core 2b holds rows q*512:q*512+256 of the
            # batch output at its own rows q*256:(q+1)*256, core 2b+1 the
            # following 256 rows of each quarter
            for q in range(4):
                out[b, q * 512:q * 512 + 256] = (
                    res[2 * b]["out"][q * 256:(q + 1) * 256].astype(np.float32)
                    + b_out)
                out[b, q * 512 + 256:(q + 1) * 512] = (
                    res[2 * b + 1]["out"][q * 256:(q + 1) * 256].astype(np.float32)
                    + b_out)
        else:
            out[b] = (res[2 * b]["out"].astype(np.float32)
                      + res[2 * b + 1]["out"].astype(np.float32) + b_out)
    return out


if __name__ == "__main__":
    rng = np.random.default_rng(0)
    x = rng.standard_normal((B, N, C)).astype(np.float32)
    w_qkv = (rng.standard_normal((3 * C, C)) * C ** -0.5).astype(np.float32)
    w_out = (rng.standard_normal((C, C)) * C ** -0.5).astype(np.float32)
    b_out = (rng.standard_normal(C) * 0.01).astype(np.float32)
    got = kernel(x=x, w_qkv=w_qkv, w_out=w_out, b_out=b_out)

    # numpy reference
    q = x @ (w_qkv[0:C]).T * SCALE
    k = x @ (w_qkv[C:2 * C]).T
    v = x @ (w_qkv[2 * C:]).T
    def heads(t):
        return t.reshape(B, N, H, HD).transpose(0, 2, 1, 3)
    qh, kh, vh = heads(q), heads(k), heads(v)
    s = np.einsum("bhnc,bhmc->bhnm", qh, kh)
    s = np.exp(s - s.max(-1, keepdims=True))
    a = s / s.sum(-1, keepdims=True)
    o = np.einsum("bhnm,bhmc->bhnc", a, vh).transpose(0, 2, 1, 3).reshape(B, N, C)
    exp = o @ w_out.T + b_out
    err = np.abs(got - exp).max() / np.abs(exp).max()
    print("rel err:", err)


# revision 23
# speedup vs baseline: 3.0373x; 1.0420x over previous
"""MHSA kernel for 8 Trainium2 NeuronCores (Bass/Tile).

Distribution (per sharding hint): data-parallel over batch (4) x
tensor-parallel over heads (2 groups of 8 heads) = 8 shards, one per core.

Per core (batch b, head-group t), on device:
  - transpose x[b] via PE -> xT (bf16)
  - qT = wqT.T @ xT, kT = wkT.T @ xT  (head-transposed layouts)
  - v  = x @ wv.T in natural [n, dv] layout, augmented with a ones column
  - per (head, 512-wide n-chunk): scoresT tiles [m=128, n=512] = kT.T @ qT,
    exp on ScalarE (no max subtraction: scores ~ N(0,1)), PV matmul with
    stationary [v_h | ones] accumulating outT[c(64)+denom(1), n] in PSUM,
    normalize by the broadcast reciprocal denominator -> vhT (bf16)
  - out_part = vhT.T @ woT  [2048, 1024] bf16
Host sums the two TP partials per batch (fp32) and adds the bias.

Executed via the bass2jax/PJRT path (axon) on cores 0-7; the compiled
sharded callable is cached module-level so repeated calls are fast.
"""
import numpy as np
import ml_dtypes

import concourse.bacc as bacc
import concourse.mybir as mybir
import concourse.tile as tile
from concourse.masks import make_identity

B, N, C, H = 4, 2048, 1024, 16
HD = C // H          # 64
NCORES = 8
TP = 2               # head groups
HPG = H // TP        # 8 heads per group
DPG = HPG * HD       # 512 dims per group
SCALE = HD ** -0.5

F32 = mybir.dt.float32
BF16 = mybir.dt.bfloat16
BF = ml_dtypes.bfloat16

_cache = {}


USE_CC = True   # on-device collectives for input reassembly / output TP-sum


def _build_nc(use_cc=None):
    if use_cc is None:
        use_cc = USE_CC
    nc = bacc.Bacc("TRN2", num_devices=NCORES)

    CS = C // 128     # 8 c-slices
    NB = N // 128     # 16 n-blocks
    NCH = N // 512    # 4 n-chunks
    DB = DPG // 128   # 4 d-blocks for qT/kT and dv-slices for vhT
    NQ = 4            # row quarters for the x-AllGather / out-ReduceScatter

    PAIRS = [[2 * b, 2 * b + 1] for b in range(B)]
    TGROUPS = [[0, 2, 4, 6], [1, 3, 5, 7]]

    if use_cc:
        # sharded inputs: each core ships 1/2 of x[b] and 1/4 of its weight
        # set; on-device AllGathers reassemble them (saves host wire bytes).
        # x is gathered in row quarters so compute overlaps the collective.
        # single packed input operand (per-operand dispatch cost is high):
        # rows [0:1024] xh, [1024:1280] wqk quarter, [1280:1408] wv quarter
        # (as [128, 1024]), [1408:1536] wo quarter
        inp_d = nc.declare_dram_parameter("inp", [3 * N // 4, C], BF16,
                                          isOutput=False)
        # output: this core's share of the TP-summed batch output, as 4
        # quarter ReduceScatters (rows q*256:(q+1)*256 of out_d = global
        # rows q*512+t*256 : q*512+(t+1)*256 of the batch output)
        out_d = nc.declare_dram_parameter("out", [N // 2, C], BF16, isOutput=True)
        xh_i = nc.dram_tensor("xh_i", [N // 2, C], BF16)
        x_d = nc.dram_tensor("x_g", [N, C], BF16)
        wqk4_i = nc.dram_tensor("wqk4_i", [C // 4, 2 * DPG], BF16)
        # wv quarter is [256, 512] semantically; declared [128, 1024] to
        # match the packed-input row shape (same bytes, row-major)
        wqk_g = nc.dram_tensor("wqk_g", [C, 2 * DPG], BF16)
        wv4_i = nc.dram_tensor("wv4_i", [C // 8, C], BF16)
        wv_g = nc.dram_tensor("wv_g", [C, DPG], BF16)
        wo4_i = nc.dram_tensor("wo4_i", [DPG // 4, C], BF16)
        wo_d = nc.dram_tensor("wo_g", [DPG, C], BF16)
        outp_q = [nc.dram_tensor(f"outp_q{q}", [N // 4, C], BF16)
                  for q in range(NQ)]
        out_rsq = [nc.dram_tensor(f"out_rs{q}", [N // 8, C], BF16)
                   for q in range(NQ)]
    else:
        x_d = nc.declare_dram_parameter("x", [N, C], BF16, isOutput=False)
        wqkv_d = nc.declare_dram_parameter("wqkvT", [C, 3 * DPG], BF16,
                                           isOutput=False)
        wo_d = nc.declare_dram_parameter("woT", [DPG, C], BF16, isOutput=False)
        out_full = nc.declare_dram_parameter("out", [N, C], BF16, isOutput=True)

    with tile.TileContext(nc) as tc:
        with (
            tc.tile_pool(name="big", bufs=1) as big,
            tc.tile_pool(name="weights", bufs=1) as wpool,
        ):
            # persistent SBUF tensors (separate tiles per slice so the Tile
            # scheduler tracks dependencies at fine grain and can overlap
            # later projections with early attention)
            xT = [big.tile([128, N], BF16, name=f"xT{cs}", tag=f"xT{cs}") for cs in range(CS)]
            qTl = [big.tile([128, N], BF16, name=f"qT{db}", tag=f"qT{db}") for db in range(DB)]
            kTl = [big.tile([128, N], BF16, name=f"kT{db}", tag=f"kT{db}") for db in range(DB)]
            v = big.tile([128, NB, HPG, 72], BF16)     # v natural + ones col @64
            vhTc = [[big.tile([128, 512], BF16, name=f"vhT{ds}_{ch}",
                              tag=f"vhT{ds}_{ch}") for ch in range(NCH)]
                    for ds in range(DB)]
            wqk_sb = [wpool.tile([128, 2 * DPG], BF16, name=f"wqk{cs}",
                                 tag=f"wqk{cs}") for cs in range(CS)]
            wv_sb = [wpool.tile([128, DPG], BF16, name=f"wv{cs}",
                                tag=f"wv{cs}") for cs in range(CS)]
            wo = wpool.tile([128, DB, C], BF16)
            ident = wpool.tile([128, 128], BF16)
            ones65 = wpool.tile([65, 64], F32)

            make_identity(nc, ident[:, :])
            nc.vector.memset(ones65[64:65, :], 1.0)
            nc.vector.memset(v[:, :, :, 64:65], 1.0)

            if use_cc:
                # reassemble sharded inputs on device; order matters: x first
                # (transposes), then wv (v-projection), then wqk, then wo
                nc.gpsimd.dma_start(out=xh_i[:], in_=inp_d[0:N // 2, :])
                nc.gpsimd.collective_compute(
                    "AllGather", mybir.AluOpType.bypass, replica_groups=PAIRS,
                    ins=[xh_i[:]], outs=[x_d[:]])
                nc.gpsimd.dma_start(out=wv4_i[:], in_=inp_d[1280:1408, :])
                nc.gpsimd.collective_compute(
                    "AllGather", mybir.AluOpType.bypass, replica_groups=TGROUPS,
                    ins=[wv4_i[:]], outs=[wv_g[:]])
                nc.gpsimd.dma_start(out=wqk4_i[:], in_=inp_d[1024:1280, :])
                nc.gpsimd.collective_compute(
                    "AllGather", mybir.AluOpType.bypass, replica_groups=TGROUPS,
                    ins=[wqk4_i[:]], outs=[wqk_g[:]])
                nc.gpsimd.dma_start(out=wo4_i[:], in_=inp_d[1408:1536, :])
                nc.gpsimd.collective_compute(
                    "AllGather", mybir.AluOpType.bypass, replica_groups=TGROUPS,
                    ins=[wo4_i[:]], outs=[wo_d[:]])
                for cs in range(CS):
                    nc.sync.dma_start(out=wv_sb[cs][:, :],
                                      in_=wv_g[cs * 128:(cs + 1) * 128, :])
                    nc.sync.dma_start(out=wqk_sb[cs][:, :],
                                      in_=wqk_g[cs * 128:(cs + 1) * 128, :])
            else:
                for cs in range(CS):
                    nc.sync.dma_start(
                        out=wv_sb[cs][:, :],
                        in_=wqkv_d[cs * 128:(cs + 1) * 128, 2 * DPG:3 * DPG])
                    nc.sync.dma_start(
                        out=wqk_sb[cs][:, :],
                        in_=wqkv_d[cs * 128:(cs + 1) * 128, 0:2 * DPG])
            for ds in range(DB):
                nc.sync.dma_start(out=wo[:, ds, :],
                                  in_=wo_d[ds * 128:(ds + 1) * 128, :])

            xsrc = [(nb, x_d, nb * 128) for nb in range(NB)]

            # ---- phase A: load x, transpose on PE, project v then q/k ----
            with (
                tc.tile_pool(name="xin", bufs=4) as xin,
                tc.tile_pool(name="tp_ps", bufs=4, space="PSUM") as tp_ps,
                tc.tile_pool(name="pj_ps", bufs=2, space="PSUM") as pj_ps,
            ):
                for nb, src, row in xsrc:
                    xn = xin.tile([128, C], BF16)
                    nc.sync.dma_start(out=xn[:, :], in_=src[row:row + 128, :])
                    for cs in range(CS):
                        tp = tp_ps.tile([128, 128], BF16)
                        nc.tensor.transpose(tp[:, :], xn[:, cs * 128:(cs + 1) * 128],
                                            ident[:, :])
                        nc.vector.tensor_copy(xT[cs][:, nb * 128:(nb + 1) * 128],
                                              tp[:, :])

                # v natural first (attention consumes all of v): [n-block 128,
                # dv 512] accumulating over c-slices
                for nb in range(NB):
                    ps = pj_ps.tile([128, HPG, 64], F32)
                    for cs in range(CS):
                        nc.tensor.matmul(
                            ps[:, :, :],
                            xT[cs][:, nb * 128:(nb + 1) * 128],
                            wv_sb[cs][:, :],
                            start=(cs == 0), stop=(cs == CS - 1))
                    nc.vector.tensor_copy(v[:, nb, :, 0:64], ps[:, :, :])

                # qT/kT per d-block (attention head h needs d-block h//2)
                for db in range(DB):
                    for qk in range(2):
                        dst = (qTl, kTl)[qk][db]
                        for ch in range(NCH):
                            ps = pj_ps.tile([128, 512], F32, tag="psqk")
                            for cs in range(CS):
                                nc.tensor.matmul(
                                    ps[:, :],
                                    wqk_sb[cs][:, qk * DPG + db * 128:
                                               qk * DPG + (db + 1) * 128],
                                    xT[cs][:, ch * 512:(ch + 1) * 512],
                                    start=(cs == 0), stop=(cs == CS - 1))
                            nc.vector.tensor_copy(
                                dst[:, ch * 512:(ch + 1) * 512], ps[:, :])

            # ---- phase B+C: attention (ch outer) with the output
            # projection and quarter ReduceScatter interleaved per n-chunk ----
            with (
                tc.tile_pool(name="st_ps", bufs=2, space="PSUM") as st_ps,
                tc.tile_pool(name="pv_ps", bufs=2, space="PSUM") as pv_ps,
                tc.tile_pool(name="mix_ps", bufs=2, space="PSUM") as mix_ps,
                tc.tile_pool(name="est", bufs=4) as est_pool,
                tc.tile_pool(name="small", bufs=6) as small,
                tc.tile_pool(name="osb", bufs=3) as osb,
            ):
                for ch in range(NCH):
                    for h in range(HPG):
                        po = (h % 2) * 64      # partition offset of this head
                        db = h // 2            # d-block holding this head
                        qh = qTl[db][po:po + 64, ch * 512:(ch + 1) * 512]
                        pv = pv_ps.tile([65, 512], F32)
                        for mb in range(0, NB, 2):
                            # two m-blocks per PSUM tile -> one 1024-wide exp
                            st = st_ps.tile([128, 2, 512], F32)
                            est = est_pool.tile([128, 2, 512], BF16)
                            for j in range(2):
                                nc.tensor.matmul(
                                    st[:, j, :],
                                    kTl[db][po:po + 64,
                                            (mb + j) * 128:(mb + j + 1) * 128],
                                    qh, start=True, stop=True)
                            nc.scalar.activation(est[:, :, :], st[:, :, :],
                                                 mybir.ActivationFunctionType.Exp)
                            for j in range(2):
                                nc.tensor.matmul(
                                    pv[:, :], v[:, mb + j, h, 0:65], est[:, j, :],
                                    start=(mb + j == 0), stop=(mb + j == NB - 1))
                        # denominator -> SBUF, broadcast over 64 partitions (PE)
                        den = small.tile([65, 512], F32)
                        nc.vector.tensor_copy(den[64:65, :], pv[64:65, :])
                        bc = mix_ps.tile([64, 512], F32, tag="mix")
                        nc.tensor.matmul(bc[:, :], ones65[64:65, :], den[64:65, :],
                                         start=True, stop=True)
                        rec = small.tile([64, 512], F32)
                        nc.vector.reciprocal_approx_fast(rec[:, :], bc[:, :])
                        vh = small.tile([64, 512], BF16)
                        nc.vector.tensor_mul(vh[:, :], pv[0:64, :], rec[:, :])
                        nc.sync.dma_start(
                            out=vhTc[db][ch][po:po + 64, :], in_=vh[:, :])

                    # output projection for this n-chunk (4 n-blocks)
                    for nb in range(4 * ch, 4 * ch + 4):
                        ot = osb.tile([128, C], BF16)
                        for dch in range(2):
                            ps = mix_ps.tile([128, 512], F32, tag="mix")
                            for ds in range(DB):
                                nc.tensor.matmul(
                                    ps[:, :],
                                    vhTc[ds][ch][:, (nb - 4 * ch) * 128:
                                                 (nb - 4 * ch + 1) * 128],
                                    wo[:, ds, dch * 512:(dch + 1) * 512],
                                    start=(ds == 0), stop=(ds == DB - 1))
                            nc.vector.tensor_copy(
                                ot[:, dch * 512:(dch + 1) * 512], ps[:, :])
                        if use_cc:
                            nc.sync.dma_start(
                                out=outp_q[ch][(nb % 4) * 128:(nb % 4 + 1) * 128, :],
                                in_=ot[:, :])
                        else:
                            nc.sync.dma_start(
                                out=out_full[nb * 128:(nb + 1) * 128, :],
                                in_=ot[:, :])
                    if use_cc:
                        # this quarter is complete: TP partial sum on device,
                        # each core keeps its half of the quarter
                        nc.gpsimd.collective_compute(
                            "ReduceScatter", mybir.AluOpType.add,
                            replica_groups=PAIRS,
                            ins=[outp_q[ch][:]], outs=[out_rsq[ch][:]])
                        nc.gpsimd.dma_start(
                            out=out_d[ch * (N // 8):(ch + 1) * (N // 8), :],
                            in_=out_rsq[ch][:])

    nc.compile()
    return nc


def _get_nc():
    if "nc" not in _cache:
        _cache["nc"] = _build_nc()
    return _cache["nc"]


def _prep(x, w_qkv, w_out):
    """Build per-core input maps. Core c -> (batch c//2, head-group c%2).

    Each core ships: half of x[b] (its pair-AllGather contribution), and a
    quarter of its head-group's weights (4-core-group AllGather).
    """
    x = np.asarray(x, dtype=np.float32)
    w_qkv = np.asarray(w_qkv, dtype=np.float32)
    w_out = np.asarray(w_out, dtype=np.float32)
    w_q, w_k, w_v = w_qkv[0:C], w_qkv[C:2 * C], w_qkv[2 * C:3 * C]
    xb = [x[b].astype(BF) for b in range(B)]
    per_t = []
    for t in range(TP):
        sl = slice(t * DPG, (t + 1) * DPG)
        wqkvT = np.concatenate(
            [(w_q[sl] * SCALE).T, w_k[sl].T, w_v[sl].T], axis=1)
        per_t.append((
            np.ascontiguousarray(wqkvT).astype(BF),        # [C, 3*DPG]
            np.ascontiguousarray(w_out[:, sl].T).astype(BF),  # [DPG, C]
        ))
    in_maps = []
    for c in range(NCORES):
        b, t = c // TP, c % TP
        g = c // 2          # index of this core within its weight group
        wqkvT, woT = per_t[t]
        if USE_CC:
            inp = np.empty((3 * N // 4, C), dtype=BF)
            inp[0:N // 2] = xb[b][t * (N // 2):(t + 1) * (N // 2)]
            inp[1024:1280] = wqkvT[g * 256:(g + 1) * 256, 0:1024]
            inp[1280:1408] = wqkvT[g * 256:(g + 1) * 256,
                                   1024:1536].reshape(128, 1024)
            inp[1408:1536] = woT[g * 128:(g + 1) * 128]
            in_maps.append({"inp": inp})
        else:
            in_maps.append({"x": xb[b], "wqkvT": wqkvT, "woT": woT})
    return in_maps


def _get_runner():
    """Persistent jitted SPMD callable over the 8 cores (built once).

    Mirrors concourse.bass2jax.run_bass_via_pjrt, but caches the jitted
    function, and does NOT pass zero buffers for the outputs: this kernel
    writes every element of its output, so the outputs can be plain
    custom-call results (saves shipping zeros per call).
    """
    if "runner" in _cache:
        return _cache["runner"]
    import jax
    import numpy as _np
    from jax.sharding import Mesh, PartitionSpec
    from jax.experimental.shard_map import shard_map
    from concourse import bass2jax, mybir as _mybir

    nc = _get_nc()
    bass2jax.install_neuronx_cc_hook()

    partition_name = nc.partition_id_tensor.name if nc.partition_id_tensor else None
    in_names, out_names, out_avals = [], [], []
    for alloc in nc.m.functions[0].allocations:
        if not isinstance(alloc, _mybir.MemoryLocationSet):
            continue
        name = alloc.memorylocations[0].name
        if alloc.kind == "ExternalInput":
            if name != partition_name:
                in_names.append(name)
        elif alloc.kind == "ExternalOutput":
            shape = tuple(alloc.tensor_shape)
            dtype = _mybir.dt.np(alloc.dtype)
            out_names.append(name)
            out_avals.append(jax.core.ShapedArray(shape, dtype))
    n_params = len(in_names)
    all_names = list(in_names)
    if partition_name is not None:
        all_names.append(partition_name)

    def _body(*args):
        operands = list(args)
        if partition_name is not None:
            operands.append(bass2jax.partition_id_tensor())
        outs = bass2jax._bass_exec_p.bind(
            *operands,
            out_avals=tuple(out_avals),
            in_names=tuple(all_names),
            out_names=tuple(out_names),
            lowering_input_output_aliases=(),
            sim_require_finite=True,
            sim_require_nnan=True,
            nc=nc,
        )
        return tuple(outs)

    devices = jax.devices()[:NCORES]
    mesh = Mesh(_np.asarray(devices), ("core",))
    sharded = jax.jit(
        shard_map(
            _body, mesh=mesh,
            in_specs=(PartitionSpec("core"),) * n_params,
            out_specs=(PartitionSpec("core"),) * len(out_names),
            check_rep=False,
        ),
        keep_unused=True,
    )

    def prepare(in_maps):
        return [
            _np.concatenate([_np.asarray(m[name]) for m in in_maps], axis=0)
            for name in in_names
        ]

    def gather(out_arrs):
        return [
            {
                name: _np.asarray(out_arrs[i]).reshape(
                    NCORES, *out_avals[i].shape)[c]
                for i, name in enumerate(out_names)
            }
            for c in range(NCORES)
        ]

    _cache["runner"] = (sharded, prepare, gather)
    return _cache["runner"]


def kernel(x, w_qkv, w_out, b_out):
    b_out = np.asarray(b_out, dtype=np.float32)
    sharded, prepare, gather = _get_runner()
    import jax
    ops = prepare(_prep(x, w_qkv, w_out))
    res = gather(jax.block_until_ready(sharded(*ops)))
    out = np.empty((B, N, C), dtype=np.float32)
    for b in range(B):
        if USE_CC:
            # quarter-RS layout: core 2b holds rows q*512:q*512+256 of the
            # batch output at its own rows q*256:(q+1)*256, core 2b+1 the
            # following 256 rows of each quarter
            for q in range(4):
                out[b, q * 512:q * 512 + 256] = (
                    res[2 * b]["out"][q * 256:(q + 1) * 256].astype(np.float32)
                    + b_out)
                out[b, q * 512 + 256:(q + 1) * 512] = (
                    res[2 * b + 1]["out"][q * 256:(q + 1) * 256].astype(np.float32)
                    + b_out)
        else:
            out[b] = (res[2 * b]["out"].astype(np.float32)
                      + res[2 * b + 1]["out"].astype(np.float32) + b_out)
    return out


if __name__ == "__main__":
    rng = np.random.default_rng(0)
    x = rng.standard_normal((B, N, C)).astype(np.float32)
    w_qkv = (rng.standard_normal((3 * C, C)) * C ** -0.5).astype(np.float32)
    w_out = (rng.standard_normal((C, C)) * C ** -0.5).astype(np.float32)
    b_out = (rng.standard_normal(C) * 0.01).astype(np.float32)
    got = kernel(x=x, w_qkv=w_qkv, w_out=w_out, b_out=b_out)

    # numpy reference
    q = x @ (w_qkv[0:C]).T * SCALE
    k = x @ (w_qkv[C:2 * C]).T
    v = x @ (w_qkv[2 * C:]).T
    def heads(t):
        return t.reshape(B, N, H, HD).transpose(0, 2, 1, 3)
    qh, kh, vh = heads(q), heads(k), heads(v)
    s = np.einsum("bhnc,bhmc->bhnm", qh, kh)
    s = np.exp(s - s.max(-1, keepdims=True))
    a = s / s.sum(-1, keepdims=True)
    o = np.einsum("bhnm,bhmc->bhnc", a, vh).transpose(0, 2, 1, 3).reshape(B, N, C)
    exp = o @ w_out.T + b_out
    err = np.abs(got - exp).max() / np.abs(exp).max()
    print("rel err:", err)
